# revision 5
# baseline (speedup 1.0000x reference)
"""Angular-select masked-FFT kernel for Trainium2 (8 NeuronCores, data-parallel over batch).

Math: per (b, g): diff[w] = sum_h ||re0|-|im1|| + ||re1|-|im0||; select 64 smallest w;
out = fft_w(ifft_h(x masked to sel columns)) + 0.5, emitted as interleaved re/im f32.

Device algorithm per (b, g):
  1. diff via ACT abs + gpsimd subtract + DVE abs-sum-reduce over free h (x fed host-transposed [w, h]).
  2. rank[w] = #{w' : d[w'] < d[w]} via DVE is_lt vs broadcast row; selected = rank < 64.
  3. sparse_gather compacts selected w ids -> int16 idx lists (shared position wrap i -> [i%16, i//16]).
  4. dma_gather pulls the 64 selected columns of x (rows of host-transposed bf16 copy) straight
     from DRAM in stage-B lhsT layout, and the 64 matching rows of the interleaved DFT const
     matrices (+ a bias row 512 that folds the final +0.5 into the matmul).
  5. stage B (ifft over h): Y^T[t, h'] accumulated over 4 h-chunks of PE matmuls vs IDFT consts.
  6. stage C (fft over w): interleaved-output matmuls Y^T @ C1/C2 -> PSUM [128, (w,2)] -> DMA to DRAM.
"""

import os
import sys
from contextlib import ExitStack

import numpy as np

sys.path.insert(0, "/opt/trn_rl_repo")

B, C, H, W = 32, 4, 512, 512
T = 64
G = 2
NCORES = 8
BPC = B // NCORES  # samples per core

_cache = {}


def _build_consts():
    import ml_dtypes

    h = np.arange(H, dtype=np.float64)
    th = 2.0 * np.pi * np.outer(h, h) / H
    cos_i = (np.cos(th) / H).astype(np.float32)
    sin_i = (np.sin(th) / H).astype(np.float32)
    w = np.arange(W, dtype=np.float64)
    tw = 2.0 * np.pi * np.outer(w, w) / W
    fr = np.cos(tw).astype(np.float32)
    fi = (-np.sin(tw)).astype(np.float32)
    # c1: coeff of Yr -> (re, im) interleaved; c2: coeff of Yi -> (re, im)
    c1 = np.zeros((W + 1, 2 * W), np.float32)
    c2 = np.zeros((W + 1, 2 * W), np.float32)
    c1[:W, 0::2] = fr
    c1[:W, 1::2] = fi
    c2[:W, 0::2] = -fi
    c2[:W, 1::2] = fr
    c1[W, 0::2] = 0.5  # bias row: +0.5 on real part only
    bf = ml_dtypes.bfloat16
    return (
        cos_i.astype(bf),
        sin_i.astype(bf),
        (-sin_i).astype(bf),
        c1.astype(bf),
        c2.astype(bf),
    )


def _build_kernel():
    import concourse.bass as bass
    import concourse.tile as tile
    from concourse import bacc, mybir
    from concourse import bass_isa

    f32 = mybir.dt.float32
    bf16 = mybir.dt.bfloat16
    i16 = mybir.dt.int16
    i32 = mybir.dt.int32
    u32 = mybir.dt.uint32
    Alu = mybir.AluOpType
    Act = mybir.ActivationFunctionType

    nc = bacc.Bacc("TRN2", target_bir_lowering=False, debug=False, num_devices=NCORES)

    xtr32 = nc.dram_tensor("xtr32", [BPC, C, W, H], f32, kind="ExternalInput").ap()
    xti32 = nc.dram_tensor("xti32", [BPC, C, W, H], f32, kind="ExternalInput").ap()
    xtr16 = nc.dram_tensor("xtr16", [BPC, C, W, H], bf16, kind="ExternalInput").ap()
    xti16 = nc.dram_tensor("xti16", [BPC, C, W, H], bf16, kind="ExternalInput").ap()
    cos_d = nc.dram_tensor("cos_i", [H, H], bf16, kind="ExternalInput").ap()
    sin_d = nc.dram_tensor("sin_i", [H, H], bf16, kind="ExternalInput").ap()
    nsin_d = nc.dram_tensor("nsin_i", [H, H], bf16, kind="ExternalInput").ap()
    c1_d = nc.dram_tensor("c1", [W + 1, 2 * W], bf16, kind="ExternalInput").ap()
    c2_d = nc.dram_tensor("c2", [W + 1, 2 * W], bf16, kind="ExternalInput").ap()
    out_d = nc.dram_tensor("out", [BPC, C, H, W, 2], f32, kind="ExternalOutput").ap()
    # scratch for bouncing per-(b,g) diff columns into a flat row
    dscr = nc.dram_tensor("dscr", [BPC, G, 4, 128], f32).ap()
    vscr = nc.dram_tensor("vscr", [BPC, G, 512], f32).ap()

    with tile.TileContext(nc) as tc, ExitStack() as ctx:
        const_pool = ctx.enter_context(tc.tile_pool(name="consts", bufs=1))
        xpool = ctx.enter_context(tc.tile_pool(name="x", bufs=2))
        apool = ctx.enter_context(tc.tile_pool(name="absd", bufs=2))
        spool = ctx.enter_context(tc.tile_pool(name="small", bufs=2))
        gpool = ctx.enter_context(tc.tile_pool(name="gather", bufs=2))
        ypool = ctx.enter_context(tc.tile_pool(name="y", bufs=2))
        psum_y = ctx.enter_context(tc.tile_pool(name="psy", bufs=2, space="PSUM"))
        psum_o = ctx.enter_context(tc.tile_pool(name="pso", bufs=2, space="PSUM"))
        opool = ctx.enter_context(tc.tile_pool(name="ostage", bufs=3))

        # --- constants in SBUF ---
        sb_cos = const_pool.tile([128, 4, H], bf16)
        sb_sin = const_pool.tile([128, 4, H], bf16)
        sb_nsin = const_pool.tile([128, 4, H], bf16)
        for hq in range(4):
            nc.sync.dma_start(sb_cos[:, hq, :], cos_d[hq * 128 : (hq + 1) * 128, :])
            nc.sync.dma_start(sb_sin[:, hq, :], sin_d[hq * 128 : (hq + 1) * 128, :])
            nc.sync.dma_start(sb_nsin[:, hq, :], nsin_d[hq * 128 : (hq + 1) * 128, :])
        iota_i = const_pool.tile([128, 4], i32)
        iota_f = const_pool.tile([128, 4], f32)
        nc.gpsimd.iota(iota_i[:], pattern=[[128, 4]], base=0, channel_multiplier=1)
        nc.vector.tensor_copy(iota_f[:], iota_i[:])

        for b in range(BPC):
            for g in range(G):
                c0, c1c = 2 * g, 2 * g + 1
                # ---------- phase 1: diff[w] ----------
                redA = spool.tile([128, 4], f32, tag="redA")
                redB = spool.tile([128, 4], f32, tag="redB")
                for wq in range(4):
                    sl = slice(wq * 128, (wq + 1) * 128)
                    t_re0 = xpool.tile([128, H], f32, tag="re0")
                    t_im1 = xpool.tile([128, H], f32, tag="im1")
                    t_re1 = xpool.tile([128, H], f32, tag="re1")
                    t_im0 = xpool.tile([128, H], f32, tag="im0")
                    nc.sync.dma_start(t_re0[:], xtr32[b, c0, sl, :])
                    nc.sync.dma_start(t_im1[:], xti32[b, c1c, sl, :])
                    nc.sync.dma_start(t_re1[:], xtr32[b, c1c, sl, :])
                    nc.sync.dma_start(t_im0[:], xti32[b, c0, sl, :])
                    a_re0 = apool.tile([128, H], f32, tag="are0")
                    a_im1 = apool.tile([128, H], f32, tag="aim1")
                    a_re1 = apool.tile([128, H], f32, tag="are1")
                    a_im0 = apool.tile([128, H], f32, tag="aim0")
                    nc.scalar.activation(a_re0[:], t_re0[:], Act.Abs)
                    nc.scalar.activation(a_im1[:], t_im1[:], Act.Abs)
                    nc.scalar.activation(a_re1[:], t_re1[:], Act.Abs)
                    nc.scalar.activation(a_im0[:], t_im0[:], Act.Abs)
                    d0 = apool.tile([128, H], f32, tag="d0")
                    d1 = apool.tile([128, H], f32, tag="d1")
                    nc.gpsimd.tensor_tensor(d0[:], a_re0[:], a_im1[:], Alu.subtract)
                    nc.gpsimd.tensor_tensor(d1[:], a_re1[:], a_im0[:], Alu.subtract)
                    nc.vector.tensor_reduce(
                        redA[:, wq : wq + 1], d0[:], mybir.AxisListType.X, Alu.add,
                        apply_absolute_value=True,
                    )
                    nc.vector.tensor_reduce(
                        redB[:, wq : wq + 1], d1[:], mybir.AxisListType.X, Alu.add,
                        apply_absolute_value=True,
                    )
                dcols = spool.tile([128, 4], f32, tag="dcols")
                nc.vector.tensor_tensor(dcols[:], redA[:], redB[:], Alu.add)
                # bounce to DRAM to relayout as a single 512-wide row
                nc.sync.dma_start(dscr[b, g].rearrange("a b -> b a"), dcols[:])
                drow1 = spool.tile([1, 512], f32, tag="drow1")
                nc.sync.dma_start(drow1[:], dscr[b, g].rearrange("a b -> (a b)"))
                drow = spool.tile([128, 512], f32, tag="drow")
                nc.gpsimd.partition_broadcast(drow[:], drow1[:])
                # ---------- phase 2: rank + select ----------
                rank = spool.tile([128, 4], f32, tag="rank")
                for wq in range(4):
                    cmp = apool.tile([128, 512], f32, tag="cmp")
                    nc.vector.tensor_scalar(
                        cmp[:], drow[:], dcols[:, wq : wq + 1], None, Alu.is_lt
                    )
                    nc.vector.tensor_reduce(
                        rank[:, wq : wq + 1], cmp[:], mybir.AxisListType.X, Alu.add
                    )
                mask = spool.tile([128, 4], mybir.dt.uint8, tag="mask")
                nc.vector.tensor_scalar(mask[:], rank[:], float(T), None, Alu.is_lt)
                vals = spool.tile([128, 4], f32, tag="vals")
                nc.vector.memset(vals[:], -1.0)
                nc.vector.copy_predicated(vals[:], mask[:], iota_f[:])
                # relayout [128,4] -> [16,32] via DRAM (any order works)
                nc.sync.dma_start(vscr[b, g].rearrange("(a b) -> a b", a=4), vals[:].rearrange("p q -> p q"))
                v16 = spool.tile([16, 32], f32, tag="v16")
                nc.sync.dma_start(v16[:], vscr[b, g].rearrange("(a b) -> a b", a=16))
                sel_f = spool.tile([16, 4], f32, tag="self")
                nfound = spool.tile([1, 1], u32, tag="nf")
                nc.gpsimd.sparse_gather(sel_f[:], v16[:], num_found=nfound[:])
                # int16 index lists (replicated across the 8 16-partition groups)
                idx_x = spool.tile([128, 8], i16, tag="idxx")
                idx_c = spool.tile([128, 8], i16, tag="idxc")
                nc.vector.memset(idx_x[:], -1)
                nc.vector.memset(idx_c[:], -1)
                nc.vector.tensor_copy(idx_x[0:16, 0:4], sel_f[:])
                nc.vector.tensor_copy(idx_c[0:16, 0:4], sel_f[:])
                nc.vector.memset(idx_c[0:1, 4:5], 512)
                for k in range(1, 8):
                    nc.sync.dma_start(idx_x[16 * k : 16 * (k + 1), :], idx_x[0:16, :])
                    nc.sync.dma_start(idx_c[16 * k : 16 * (k + 1), :], idx_c[0:16, :])
                # ---------- phase 3: gathers ----------
                gc1 = gpool.tile([128, 1, 2 * W], bf16, tag="gc1")
                gc2 = gpool.tile([128, 1, 2 * W], bf16, tag="gc2")
                nc.gpsimd.dma_gather(
                    gc1[:], c1_d[:], idx_c[:, 0:5], num_idxs=80, num_idxs_reg=65,
                    elem_size=2 * W,
                )
                nc.gpsimd.dma_gather(
                    gc2[:], c2_d[:], idx_c[:, 0:5], num_idxs=80, num_idxs_reg=65,
                    elem_size=2 * W,
                )
                for cc in (c0, c1c):
                    gxr = gpool.tile([128, 4, 128], bf16, tag="gxr")
                    gxi = gpool.tile([128, 4, 128], bf16, tag="gxi")
                    nc.gpsimd.dma_gather(
                        gxr[:], xtr16[b, cc], idx_x[:], num_idxs=128, num_idxs_reg=64,
                        elem_size=H, transpose=True,
                    )
                    nc.gpsimd.dma_gather(
                        gxi[:], xti16[b, cc], idx_x[:], num_idxs=128, num_idxs_reg=64,
                        elem_size=H, transpose=True,
                    )
                    # ---------- phase 4: stage B (ifft over h) ----------
                    yr_ps = psum_y.tile([64, H], f32, tag="yrp")
                    yi_ps = psum_y.tile([64, H], f32, tag="yip")
                    for hq in range(4):
                        first = hq == 0
                        last = hq == 3
                        nc.tensor.matmul(
                            yr_ps[:], gxr[:, hq, 0:T], sb_cos[:, hq, :],
                            start=first, stop=False,
                        )
                        nc.tensor.matmul(
                            yr_ps[:], gxi[:, hq, 0:T], sb_nsin[:, hq, :],
                            start=False, stop=last,
                        )
                        nc.tensor.matmul(
                            yi_ps[:], gxr[:, hq, 0:T], sb_sin[:, hq, :],
                            start=first, stop=False,
                        )
                        nc.tensor.matmul(
                            yi_ps[:], gxi[:, hq, 0:T], sb_cos[:, hq, :],
                            start=False, stop=last,
                        )
                    yr_sb = ypool.tile([65, H], bf16, tag="yr")
                    yi_sb = ypool.tile([65, H], bf16, tag="yi")
                    nc.vector.tensor_copy(yr_sb[0:64, :], yr_ps[:])
                    nc.vector.tensor_copy(yi_sb[0:64, :], yi_ps[:])
                    nc.vector.memset(yr_sb[64:65, :], 1.0)
                    nc.vector.memset(yi_sb[64:65, :], 0.0)
                    # ---------- phase 5: stage C (fft over w, interleaved out) ----------
                    for mq in range(4):
                        msl = slice(mq * 128, (mq + 1) * 128)
                        o_ps = psum_o.tile([128, 2 * W], f32, tag="ops")
                        for nh in range(2):
                            nsl = slice(nh * W, (nh + 1) * W)
                            nc.tensor.matmul(
                                o_ps[:, nsl], yr_sb[:, msl], gc1[0:65, 0, nsl],
                                start=True, stop=False,
                            )
                            nc.tensor.matmul(
                                o_ps[:, nsl], yi_sb[:, msl], gc2[0:65, 0, nsl],
                                start=False, stop=True,
                            )
                        o_sb = opool.tile([128, 2 * W], f32, tag="osb")
                        if mq % 2 == 0:
                            nc.vector.tensor_copy(o_sb[:], o_ps[:])
                        else:
                            nc.scalar.mul(o_sb[:], o_ps[:], 1.0)
                        nc.sync.dma_start(
                            out_d[b, cc, msl].rearrange("p a b -> p (a b)"), o_sb[:]
                        )

    nc.compile()
    return nc


def _get_nc():
    if "nc" not in _cache:
        _cache["nc"] = _build_kernel()
    return _cache["nc"]


def kernel(x_real: np.ndarray, x_imag: np.ndarray) -> np.ndarray:
    import ml_dtypes
    from concourse.bass_utils import run_bass_kernel_spmd

    bf = ml_dtypes.bfloat16
    xr = np.ascontiguousarray(np.asarray(x_real, dtype=np.float32))
    xi = np.ascontiguousarray(np.asarray(x_imag, dtype=np.float32))
    xtr = np.ascontiguousarray(xr.transpose(0, 1, 3, 2))  # [B, C, W, H]
    xti = np.ascontiguousarray(xi.transpose(0, 1, 3, 2))
    xtr16 = xtr.astype(bf)
    xti16 = xti.astype(bf)
    cos_i, sin_i, nsin_i, c1, c2 = _cache.setdefault("consts", _build_consts())

    nc = _get_nc()
    in_maps = []
    for i in range(NCORES):
        sl = slice(i * BPC, (i + 1) * BPC)
        in_maps.append(
            {
                "xtr32": xtr[sl],
                "xti32": xti[sl],
                "xtr16": xtr16[sl],
                "xti16": xti16[sl],
                "cos_i": cos_i,
                "sin_i": sin_i,
                "nsin_i": nsin_i,
                "c1": c1,
                "c2": c2,
            }
        )
    res = run_bass_kernel_spmd(nc, in_maps, core_ids=list(range(NCORES)))
    outs = [res.results[i]["out"] for i in range(NCORES)]
    return np.concatenate(outs, axis=0)


if __name__ == "__main__":
    rng = np.random.RandomState(0)
    out = kernel(
        rng.randn(B, C, H, W).astype(np.float32),
        rng.randn(B, C, H, W).astype(np.float32),
    )
    print(out.shape, out.dtype)


# revision 43
# speedup vs baseline: 1.4738x; 1.4738x over previous
"""Angular-select masked-FFT kernel for Trainium2 (8 NeuronCores, data-parallel over batch).

Math: per (b, g): diff[w] = sum_h ||re0|-|im1|| + ||re1|-|im0||; select 64 smallest w;
out = fft_w(ifft_h(x masked to sel columns)) + 0.5, emitted as interleaved re/im f32.

Device algorithm per (b, g) unit, software-pipelined (phase2 lags phase1 by 2 units):
  phase 1 (select):
    - diff: one [128, 4x512] in-place elementwise chain (ACT/DVE/gpsimd), then
      PE ones-matmul reduces h -> PSUM [1, 512] diff row.
    - rank[w] = #{w': d[w'] < d[w]} via fused is_lt+accum DVE ops against the
      partition-broadcast row; selected = rank < 64 (exactly 64, no ties).
    - sparse_gather compacts selected w ids; int16 idx lists wrap position i -> [i%16, i//16].
    - one dma_gather pulls all 4 tensors' selected columns (256 rows of the stacked
      host-transposed bf16 copy) in stage-B lhsT layout; two more pull the matching
      rows of the interleaved DFT const matrices (+ bias row folding +0.5 in).
  phase 2 (FFTs as matmuls):
    - stage B (ifft over h): Y^T[t, h'] += lhsT-gathered X vs IDFT consts (PE, bf16).
    - stage C (fft over w): interleaved-output matmuls Y^T @ C1/C2 -> PSUM [128, (w,2)]
      -> evict (DVE/ACT alternating) -> contiguous DMA to DRAM.
"""

import os
import sys
from contextlib import ExitStack

import numpy as np

sys.path.insert(0, "/opt/trn_rl_repo")

B, C, H, W = 32, 4, 512, 512
T = 64
G = 2
NCORES = 8
BPC = B // NCORES  # samples per core

_cache = {}


def _build_consts():
    import ml_dtypes

    h = np.arange(H, dtype=np.float64)
    th = 2.0 * np.pi * np.outer(h, h) / H
    cos_i = (np.cos(th) / H).astype(np.float32)
    sin_i = (np.sin(th) / H).astype(np.float32)
    w = np.arange(W, dtype=np.float64)
    tw = 2.0 * np.pi * np.outer(w, w) / W
    fr = np.cos(tw).astype(np.float32)
    fi = (-np.sin(tw)).astype(np.float32)
    # c1: coeff of Yr -> (re, im) interleaved; c2: coeff of Yi -> (re, im)
    c1 = np.zeros((W + 1, 2 * W), np.float32)
    c2 = np.zeros((W + 1, 2 * W), np.float32)
    c1[:W, 0::2] = fr
    c1[:W, 1::2] = fi
    c2[:W, 0::2] = -fi
    c2[:W, 1::2] = fr
    c1[W, 0::2] = 0.5  # bias row: +0.5 on real part only
    bf = ml_dtypes.bfloat16
    cboth = np.concatenate([c1, c2], axis=0).astype(bf)  # [1026, 1024]
    return cos_i.astype(bf), sin_i.astype(bf), (-sin_i).astype(bf), cboth


def _build_kernel():
    import concourse.bass as bass
    import concourse.tile as tile
    from concourse import bacc, mybir

    f32 = mybir.dt.float32
    bf16 = mybir.dt.bfloat16
    i16 = mybir.dt.int16
    i32 = mybir.dt.int32
    u32 = mybir.dt.uint32
    u8 = mybir.dt.uint8
    Alu = mybir.AluOpType
    Act = mybir.ActivationFunctionType
    MASK = 0x7FFFFFFF

    nc = bacc.Bacc("TRN2", target_bir_lowering=False, debug=False, num_devices=NCORES)

    xr32 = nc.dram_tensor("xr32", [BPC, C, H, W], f32, kind="ExternalInput").ap()
    xi32 = nc.dram_tensor("xi32", [BPC, C, H, W], f32, kind="ExternalInput").ap()
    # stacked transposed bf16: row ((c*2+ri)*512 + w) -> x[b,c,:,w] (re/im by ri)
    x16 = nc.dram_tensor("x16", [BPC, C * 2 * W, H], bf16, kind="ExternalInput").ap()
    cos_d = nc.dram_tensor("cos_i", [H, H], bf16, kind="ExternalInput").ap()
    sin_d = nc.dram_tensor("sin_i", [H, H], bf16, kind="ExternalInput").ap()
    nsin_d = nc.dram_tensor("nsin_i", [H, H], bf16, kind="ExternalInput").ap()
    cb_d = nc.dram_tensor("cboth", [2 * (W + 1), 2 * W], bf16, kind="ExternalInput").ap()
    out_d = nc.dram_tensor("out", [BPC, C, H, W, 2], f32, kind="ExternalOutput").ap()
    dscr = nc.dram_tensor("dscr", [BPC, G, 512], f32).ap()
    vscr = nc.dram_tensor("vscr", [BPC, G, 512], f32).ap()

    with tile.TileContext(nc) as tc, ExitStack() as ctx:
        const_pool = ctx.enter_context(tc.tile_pool(name="consts", bufs=1))
        xpool = ctx.enter_context(tc.tile_pool(name="x", bufs=2))
        spool = ctx.enter_context(tc.tile_pool(name="small", bufs=8))
        bpool = ctx.enter_context(tc.tile_pool(name="brow", bufs=3))
        gpool = ctx.enter_context(tc.tile_pool(name="gather", bufs=8))
        ypool = ctx.enter_context(tc.tile_pool(name="y", bufs=3))
        psum_d = ctx.enter_context(tc.tile_pool(name="psd", bufs=2, space="PSUM"))
        psum_y = ctx.enter_context(tc.tile_pool(name="psy", bufs=1, space="PSUM"))
        psum_o = ctx.enter_context(tc.tile_pool(name="pso", bufs=2, space="PSUM"))
        opool = ctx.enter_context(tc.tile_pool(name="ostage", bufs=6))

        # --- constants in SBUF ---
        sb_cos = const_pool.tile([128, 4, H], bf16)
        sb_sin = const_pool.tile([128, 4, H], bf16)
        sb_nsin = const_pool.tile([128, 4, H], bf16)
        nc.sync.dma_start(sb_cos[:], cos_d.rearrange("(a p) w -> p a w", p=128))
        nc.sync.dma_start(sb_sin[:], sin_d.rearrange("(a p) w -> p a w", p=128))
        nc.sync.dma_start(sb_nsin[:], nsin_d.rearrange("(a p) w -> p a w", p=128))
        iota_i = const_pool.tile([128, 4], i32)
        iota_f = const_pool.tile([128, 4], f32)
        nc.gpsimd.iota(iota_i[:], pattern=[[128, 4]], base=0, channel_multiplier=1)
        nc.vector.tensor_copy(iota_f[:], iota_i[:])
        ones_t = const_pool.tile([128, 1], f32)
        nc.vector.memset(ones_t[:], 1.0)

        state = {}

        def phase1(b, g):
            c0, c1c = 2 * g, 2 * g + 1
            # ---- diff[w]: full-width in-place elementwise + PE h-reduce ----
            diff_ps = psum_d.tile([1, W], f32, tag="dps")
            t_re0 = xpool.tile([128, 4, W], f32, tag="re0")
            t_im1 = xpool.tile([128, 4, W], f32, tag="im1")
            t_re1 = xpool.tile([128, 4, W], f32, tag="re1")
            t_im0 = xpool.tile([128, 4, W], f32, tag="im0")
            nc.sync.dma_start(t_re0[:], xr32[b, c0].rearrange("(a p) w -> p a w", p=128))
            nc.sync.dma_start(t_im1[:], xi32[b, c1c].rearrange("(a p) w -> p a w", p=128))
            nc.sync.dma_start(t_re1[:], xr32[b, c1c].rearrange("(a p) w -> p a w", p=128))
            nc.sync.dma_start(t_im0[:], xi32[b, c0].rearrange("(a p) w -> p a w", p=128))
            nc.scalar.activation(t_re0[:], t_re0[:], Act.Abs)
            nc.vector.tensor_scalar(
                t_im1[:].bitcast(i32), t_im1[:].bitcast(i32), MASK, None,
                Alu.bitwise_and,
            )
            nc.scalar.activation(t_re1[:], t_re1[:], Act.Abs)
            nc.vector.tensor_scalar(
                t_im0[:].bitcast(i32), t_im0[:].bitcast(i32), MASK, None,
                Alu.bitwise_and,
            )
            nc.gpsimd.tensor_tensor(t_re0[:], t_re0[:], t_im1[:], Alu.subtract)
            nc.vector.tensor_tensor(t_re1[:], t_re1[:], t_im0[:], Alu.subtract)
            nc.scalar.activation(t_re0[:], t_re0[:], Act.Abs)
            nc.vector.tensor_scalar(
                t_re1[:].bitcast(i32), t_re1[:].bitcast(i32), MASK, None,
                Alu.bitwise_and,
            )
            nc.vector.tensor_tensor(t_re0[:], t_re0[:], t_re1[:], Alu.add)
            for hq in range(4):
                nc.tensor.matmul(
                    diff_ps[:], ones_t[:, 0:1], t_re0[:, hq, :],
                    start=(hq == 0), stop=(hq == 3),
                )
            # ---- rank + select ----
            drow1 = spool.tile([1, W], f32, tag="drow1")
            nc.vector.tensor_copy(drow1[:], diff_ps[:])
            drow = bpool.tile([128, W], f32, tag="drow")
            nc.gpsimd.partition_broadcast(drow[:], drow1[:])
            nc.sync.dma_start(
                dscr[b, g].rearrange("(a b) -> a b", a=1), drow1[0:1, :]
            )
            dcol = spool.tile([128, 4], f32, tag="dcol")
            nc.sync.dma_start(dcol[:], dscr[b, g].rearrange("(a b) -> b a", b=128))
            rank = spool.tile([128, 4], f32, tag="rank")
            cmp = bpool.tile([128, W], f32, tag="cmp")
            for wq in range(4):
                nc.vector.tensor_scalar(
                    cmp[:], drow[:], dcol[:, wq : wq + 1], 0.0, Alu.is_lt,
                    Alu.add, accum_out=rank[:, wq : wq + 1],
                )
            mask = spool.tile([128, 4], u8, tag="mask")
            nc.vector.tensor_scalar(mask[:], rank[:], float(T), None, Alu.is_lt)
            vals = spool.tile([128, 4], f32, tag="vals")
            nc.vector.memset(vals[:], -1.0)
            nc.vector.copy_predicated(vals[:], mask[:], iota_f[:])
            nc.sync.dma_start(vscr[b, g].rearrange("(a b) -> b a", b=128), vals[:])
            v16 = spool.tile([16, 32], f32, tag="v16")
            nc.sync.dma_start(v16[:], vscr[b, g].rearrange("(a b) -> a b", a=16))
            sel_f = spool.tile([16, 4], f32, tag="self")
            nfound = spool.tile([1, 1], u32, tag="nf")
            nc.gpsimd.sparse_gather(sel_f[:], v16[:], num_found=nfound[:])
            sel16 = spool.tile([16, 4], i16, tag="sel16")
            nc.vector.tensor_copy(sel16[:], sel_f[:])
            # ---- index lists + gathers ----
            idx_a = spool.tile([128, 21], i16, tag="idxa")
            nc.vector.memset(idx_a[0:16, :], -1)
            for j in range(4):
                off = (4 * g + j) * W
                nc.vector.tensor_scalar(
                    idx_a[0:16, 4 * j : 4 * j + 4], sel16[:], off, None, Alu.add
                )
            nc.vector.tensor_copy(idx_a[0:16, 16:20], sel16[:])
            nc.vector.memset(idx_a[0:1, 20:21], W)
            nc.sync.dma_start(idx_a[16:32, :], idx_a[0:16, :])
            nc.sync.dma_start(idx_a[32:64, :], idx_a[0:32, :])
            nc.sync.dma_start(idx_a[64:128, :], idx_a[0:64, :])
            gx = gpool.tile([128, 4, 256], bf16, tag="gx")
            gc = gpool.tile([128, 2, 2 * W], bf16, tag="gc")
            nc.gpsimd.dma_gather(
                gx[:], x16[b], idx_a[:, 0:16], num_idxs=256, num_idxs_reg=256,
                elem_size=H, transpose=True,
            )
            nc.gpsimd.dma_gather(
                gc[:, 0:1, :], cb_d[0 : W + 1, :], idx_a[:, 16:21],
                num_idxs=80, num_idxs_reg=65, elem_size=2 * W,
            )
            nc.gpsimd.dma_gather(
                gc[:, 1:2, :], cb_d[W + 1 :, :], idx_a[:, 16:21],
                num_idxs=80, num_idxs_reg=65, elem_size=2 * W,
            )
            state[(b, g)] = (c0, c1c, gx, gc)

        def phase2(b, g):
            c0, c1c, gx, gc = state.pop((b, g))
            for ci, cc in enumerate((c0, c1c)):
                jr, ji = 2 * ci, 2 * ci + 1
                # ---- stage B (ifft over h) ----
                yr_ps = psum_y.tile([64, H], f32, tag="yrp")
                yi_ps = psum_y.tile([64, H], f32, tag="yip")
                for hq in range(4):
                    first = hq == 0
                    last = hq == 3
                    lr = gx[:, hq, 64 * jr : 64 * jr + T]
                    li = gx[:, hq, 64 * ji : 64 * ji + T]
                    nc.tensor.matmul(
                        yr_ps[:], lr, sb_cos[:, hq, :], start=first, stop=False
                    )
                    nc.tensor.matmul(
                        yr_ps[:], li, sb_nsin[:, hq, :], start=False, stop=last
                    )
                    nc.tensor.matmul(
                        yi_ps[:], lr, sb_sin[:, hq, :], start=first, stop=False
                    )
                    nc.tensor.matmul(
                        yi_ps[:], li, sb_cos[:, hq, :], start=False, stop=last
                    )
                yr_sb = ypool.tile([65, H], bf16, tag="yr")
                yi_sb = ypool.tile([65, H], bf16, tag="yi")
                nc.vector.tensor_copy(yr_sb[0:64, :], yr_ps[:])
                nc.vector.tensor_copy(yi_sb[0:64, :], yi_ps[:])
                nc.vector.memset(yr_sb[64:65, :], 1.0)
                nc.vector.memset(yi_sb[64:65, :], 0.0)
                # ---- stage C (fft over w, interleaved out) ----
                for mq in range(4):
                    msl = slice(mq * 128, (mq + 1) * 128)
                    o_ps = psum_o.tile([128, 2 * W], f32, tag="ops")
                    for nh in range(2):
                        nsl = slice(nh * W, (nh + 1) * W)
                        nc.tensor.matmul(
                            o_ps[:, nsl], yr_sb[:, msl], gc[0:65, 0, nsl],
                            start=True, stop=False,
                        )
                        nc.tensor.matmul(
                            o_ps[:, nsl], yi_sb[:, msl], gc[0:65, 1, nsl],
                            start=False, stop=True,
                        )
                    o_sb = opool.tile([128, 2 * W], f32, tag="osb")
                    if mq % 2 == 0:
                        nc.vector.tensor_copy(o_sb[:], o_ps[:])
                    else:
                        nc.scalar.mul(o_sb[:], o_ps[:], 1.0)
                    nc.sync.dma_start(
                        out_d[b, cc, msl].rearrange("p a b -> p (a b)"), o_sb[:]
                    )

        units = [(b, g) for b in range(BPC) for g in range(G)]
        LOOKAHEAD = 2
        for k in range(len(units) + LOOKAHEAD):
            if k < len(units):
                phase1(*units[k])
            if k >= LOOKAHEAD:
                phase2(*units[k - LOOKAHEAD])

    nc.compile()
    return nc


def _get_nc():
    if "nc" not in _cache:
        _cache["nc"] = _build_kernel()
    return _cache["nc"]


def _make_in_maps(xr, xi):
    import ml_dtypes

    bf = ml_dtypes.bfloat16
    cos_i, sin_i, nsin_i, cboth = _cache.setdefault("consts", _build_consts())
    # stacked transposed bf16 copy: [B, C, 2, W, H] -> rows (c*2+ri)*W + w
    x16 = np.stack(
        [xr.transpose(0, 1, 3, 2), xi.transpose(0, 1, 3, 2)], axis=2
    ).astype(bf)
    x16 = np.ascontiguousarray(x16).reshape(B, C * 2 * W, H)
    in_maps = []
    for i in range(NCORES):
        sl = slice(i * BPC, (i + 1) * BPC)
        in_maps.append(
            {
                "xr32": xr[sl],
                "xi32": xi[sl],
                "x16": x16[sl],
                "cos_i": cos_i,
                "sin_i": sin_i,
                "nsin_i": nsin_i,
                "cboth": cboth,
            }
        )
    return in_maps


def kernel(x_real: np.ndarray, x_imag: np.ndarray) -> np.ndarray:
    from concourse.bass_utils import run_bass_kernel_spmd

    xr = np.ascontiguousarray(np.asarray(x_real, dtype=np.float32))
    xi = np.ascontiguousarray(np.asarray(x_imag, dtype=np.float32))
    nc = _get_nc()
    in_maps = _make_in_maps(xr, xi)
    res = run_bass_kernel_spmd(nc, in_maps, core_ids=list(range(NCORES)))
    outs = [res.results[i]["out"] for i in range(NCORES)]
    return np.concatenate(outs, axis=0)


if __name__ == "__main__":
    rng = np.random.RandomState(0)
    out = kernel(
        rng.randn(B, C, H, W).astype(np.float32),
        rng.randn(B, C, H, W).astype(np.float32),
    )
    print(out.shape, out.dtype)
